# revision 20
# baseline (speedup 1.0000x reference)
"""Trainium2 Bass kernel for nn_CrossModalAttention (sparse per-channel 3x3
token-window attention).

Contract: kernel(**inputs) takes the FULL fp32 inputs (B=8,C=256,H=W=64) and
returns the FULL fp32 output.  Internally: data-parallel over batch across the
8 NeuronCores (1 batch element per core), params replicated.

Layout: everything on-chip is d-major — activations are [c, d, t] where d is
the pixel-within-token index (16) and t = (I,J) the token index (256).  This
makes every DVE elementwise op innermost-contiguous over t (bf16 2x mode),
makes the per-d / per-n PE reduction matmuls read contiguous columns, and
turns the attention-weight broadcast over d into a free outer-dim 0-stride.

v2 schedule (vs v1):
  - inputs DMA'd in column chunks; Q/K projections interleaved per u-pair so
    the DVE QK products start as soon as the first half of Q/K lands.
  - DVE program order: QKprod(g0), QKprod(g1), softmax(g0), AVprod(g0),
    softmax(g1), AVprod(g1) — the DVE never waits on the PE mid-stream.
  - unpadded V copy (vun) gives all 9 AV products DVE 2x mode (v1: 6 of 9).
  - softmax: pairwise tree partly in bf16, reciprocal_approx_fast, recip
    cast to bf16 on the scalar engine.
  - residual add folded into the PE's AV accumulation (identity-weight matmul
    of blue into PSUM); ACT evacuates straight to bf16.
  - output stored as bf16 (host casts back to f32) — halves the store tail.
"""

import os
import sys
from contextlib import ExitStack

import numpy as np

for _p in ("/opt/trn_rl_repo",):
    if _p not in sys.path and os.path.isdir(_p):
        sys.path.insert(0, _p)

import ml_dtypes  # noqa: E402

import concourse.bacc as bacc  # noqa: E402
import concourse.bass as bass  # noqa: E402
import concourse.tile as tile  # noqa: E402
from concourse import mybir  # noqa: E402
from concourse.bass_utils import run_bass_kernel_spmd  # noqa: E402

BF16 = mybir.dt.bfloat16
F32 = mybir.dt.float32
ALU = mybir.AluOpType
ACTF = mybir.ActivationFunctionType

B, C, H, W = 8, 256, 64, 64
TS = 4                      # token size
NH = H // TS                # 16 token rows
NW = W // TS                # 16 token cols
T = NH * NW                 # 256 tokens
D = TS * TS                 # 16 pixels per token
G = 2                       # channel groups of 128
P = 128
PIX = H * W                 # 4096
GRID = NH + 2               # 18 (zero-padded token grid)
SCALE = float(D) ** -0.5    # 0.25
N_CORES = 8
CH = 1024                   # proj psum chunk: 4 d-planes x 256 t

_BUILT = None


def _emit(ctx: ExitStack, tc: "tile.TileContext"):
    nc = tc.nc

    # ---- DRAM I/O (per-core shard); activations d-major [p, g, d, t] ----
    xb_d = nc.dram_tensor("xb", [P, G, PIX], BF16, kind="ExternalInput").ap()
    xw_d = nc.dram_tensor("xw", [P, G, PIX], BF16, kind="ExternalInput").ap()
    sm_d = nc.dram_tensor("smalls", [P, 1670], BF16, kind="ExternalInput").ap()
    out_d = nc.dram_tensor("out", [P, G, PIX], BF16, kind="ExternalOutput").ap()

    consts = ctx.enter_context(tc.tile_pool(name="consts", bufs=1))

    # ---- persistent SBUF tiles ----
    xb = consts.tile([P, G, PIX], BF16)          # blue, d-major token order
    sm = consts.tile([P, 1670], BF16)            # packed: ident | W^T | biases
    ident = sm[:, 0:P]
    wall = sm[:, P:P + 3 * G * C].rearrange("p (w g c) -> p w g c", w=3, g=G)
    ball = sm[:, P + 3 * G * C:]                 # bf16 biases, col = proj*2+g
    xwpool = tc.tile_pool(name="xwp", bufs=1)
    xw = xwpool.__enter__().tile([P, G, PIX], BF16, tag="xw")  # proj phase only
    qsb = consts.tile([P, G, D, T], BF16)
    # K/V each as three flat copies: center, shift+1 (k1[t]=K[t-1], dj=0 view)
    # and shift-1 (k2[t]=K[t+1], dj=2 view).  k1/k2 are built from kun by one
    # contiguous SBUF->SBUF DMA; J-edge columns re-zeroed by small memsets.
    # All neighbor views are then even-offset flat slices -> DVE 2x mode.
    kun = consts.tile([P, G, D, T], BF16)
    k1 = consts.tile([P, G, D, T], BF16)
    k2 = consts.tile([P, G, D, T], BF16)
    vun = consts.tile([P, G, D, T], BF16)
    v1 = consts.tile([P, G, D, T], BF16)
    v2 = consts.tile([P, G, D, T], BF16)
    esb = consts.tile([P, G, 9, T], BF16)        # exp(logits), slot-major
    tA = consts.tile([P, 4, T], BF16)            # softmax tree scratch
    tB = consts.tile([P, 2, T], F32)
    ssum = consts.tile([P, 2, T], F32)           # [0]=sum, [1]=recip (per g reuse)
    rb = consts.tile([P, G, T], BF16)            # recip, bf16 for 2x scale

    # input DMA, column-chunked so projections can start on chunk 0
    nc.sync.dma_start(sm[:], sm_d[:])
    for c in range(PIX // CH):
        cs = slice(c * CH, (c + 1) * CH)
        nc.sync.dma_start(xb[:, :, cs], xb_d[:, :, cs])
        nc.sync.dma_start(xw[:, :, cs], xw_d[:, :, cs])

    # ---- projections: out[c, d, t] = sum_a W[c,a] x[a, d, t] + b[c] ----
    with tc.tile_pool(name="psumP", bufs=2, space="PSUM") as psum:
        # emission plan: Q/K interleaved per u-pair (QK products can start
        # after u0,u1 of Q+K for a g), V projections last.
        plan = []
        for g in range(G):
            for up in range(2):
                for proj in (0, 1):
                    plan.append((proj, g, 2 * up))
                    plan.append((proj, g, 2 * up + 1))
        for g in range(G):
            for u in range(4):
                plan.append((2, g, u))

        for proj, g, u in plan:
            src = xb if proj == 0 else xw
            bias_ap = ball[:, proj * 2 + g: proj * 2 + g + 1]
            pt = psum.tile([P, CH], F32, tag="psP")
            for h in range(2):  # h-outer: one weight load per half
                for j in range(CH // 512):
                    cols = slice(u * CH + j * 512, u * CH + (j + 1) * 512)
                    mm = nc.tensor.matmul(
                        pt[:, j * 512:(j + 1) * 512],
                        wall[:, proj, h, g * P:(g + 1) * P],
                        src[:, h, cols],
                        start=(h == 0),
                        stop=(h == 1),
                    )
                    if j > 0:
                        mm.ins.ldweights = False
            ds = slice(4 * u, 4 * u + 4)
            dst = (qsb, kun, vun)[proj]
            nc.scalar.activation(
                dst[:, g, ds, :], pt[:], ACTF.Identity, bias=bias_ap
            )
            if proj > 0 and u % 2 == 1:  # shifted copies per dh half (2048)
                un, s1, s2 = (kun, k1, k2) if proj == 1 else (vun, v1, v2)
                dh = u // 2
                a, b = dh * 2048, dh * 2048 + 2048
                uf, s1f, s2f = un[:, g], s1[:, g], s2[:, g]
                uf = uf.rearrange("p d t -> p (d t)")
                s1f = s1f.rearrange("p d t -> p (d t)")
                s2f = s2f.rearrange("p d t -> p (d t)")
                nc.sync.dma_start(s1f[:, max(a, 1):b], uf[:, max(a, 1) - 1:b - 1])
                nc.sync.dma_start(
                    s2f[:, a:min(b, PIX - 1)], uf[:, a + 1:min(b, PIX - 1) + 1]
                )

    xwpool.__exit__(None, None, None)  # free xw; attention pools reuse it
    prod = ctx.enter_context(tc.tile_pool(name="prod", bufs=3))
    avp = ctx.enter_context(tc.tile_pool(name="avp", bufs=3))
    outp = ctx.enter_context(tc.tile_pool(name="outf", bufs=2))

    # ---- attention ----
    # neighbor n = 3*di + dj, di,dj in {0,1,2}.  dj selects the flat copy
    # (k1 / kun / k2); di is a +-16 token shift with product-edge memsets.
    def jzero(t3, g):
        """Zero the J-edge columns of a shifted copy (after its DMA)."""
        col = 0 if t3 in (k1, v1) else NW - 1
        nc.vector.memset(t3[:, g, :, col:T:NW], 0.0)

    first_ident = [True]

    def imm(out_ap, mov_ap, start, stop):
        m = nc.tensor.matmul(out_ap, ident, mov_ap, start=start, stop=stop)
        if first_ident[0]:
            first_ident[0] = False
        else:
            m.ins.ldweights = False
        return m

    # --- QK phase: products (DVE), d-reduction (PE), exp (ACT) ---
    with tc.tile_pool(name="psumL", bufs=2, space="PSUM") as psL:
        for g in range(G):
            jzero(k1, g)
            jzero(k2, g)
            qv = qsb[:, g]  # [P, D, T]
            for di in range(3):          # group = token row di, slots 3di+dj
                halves = []
                for dh in range(2):
                    ph = prod.tile([P, 3, D // 2, T], BF16, tag="prod")
                    dsl = slice(8 * dh, 8 * dh + 8)
                    if di == 0:
                        nc.vector.memset(ph[:, :, :, 0:16], 0.0)
                    elif di == 2:
                        nc.vector.memset(ph[:, :, :, T - 16:T], 0.0)
                    for dj, src in enumerate((k1, kun, k2)):
                        kf = src[:, g, dsl, :]
                        if di == 1:
                            nc.vector.tensor_tensor(
                                ph[:, dj], qv[:, dsl, :], kf, op=ALU.mult,
                            )
                        elif di == 0:  # K row I-1: valid t >= 16
                            nc.vector.tensor_tensor(
                                ph[:, dj, :, 16:T], qv[:, dsl, 16:T],
                                kf[:, :, 0:T - 16], op=ALU.mult,
                            )
                        else:  # di == 2: K row I+1: valid t < 240
                            nc.vector.tensor_tensor(
                                ph[:, dj, :, 0:T - 16], qv[:, dsl, 0:T - 16],
                                kf[:, :, 16:T], op=ALU.mult,
                            )
                    halves.append(ph)
                lp = psL.tile([P, 768], F32, tag="psL")
                for d in range(D):  # accumulate over d on PE (identity weights)
                    imm(lp[:, 0:512], halves[d // 8][:, 0:2, d % 8, :],
                        start=(d == 0), stop=(d == D - 1))
                    imm(lp[:, 512:768], halves[d // 8][:, 2:3, d % 8, :],
                        start=(d == 0), stop=(d == D - 1))
                nc.scalar.activation(  # e = exp(scale*l) -> [slot, t]
                    esb[:, g, 3 * di:3 * di + 3, :],
                    lp[:], ACTF.Exp, scale=SCALE,
                )

    # --- softmax normalize + AV phase ---
    psA = ctx.enter_context(tc.tile_pool(name="psumA", bufs=2, space="PSUM"))
    for g in range(G):
        jzero(v1, g)
        jzero(v2, g)
        # denominator: pairwise tree (bf16 first level, then fp32)
        nc.vector.tensor_tensor(
            tA[:], esb[:, g, 0:4, :], esb[:, g, 4:8, :], op=ALU.add
        )
        nc.vector.tensor_tensor(tB[:], tA[:, 0:2, :], tA[:, 2:4, :], op=ALU.add)
        nc.vector.tensor_tensor(
            ssum[:, 0, :], tB[:, 0, :], tB[:, 1, :], op=ALU.add
        )
        nc.vector.tensor_tensor(
            ssum[:, 1, :], ssum[:, 0, :], esb[:, g, 8, :], op=ALU.add
        )
        nc.vector.reciprocal_approx_fast(ssum[:, 0, :], ssum[:, 1, :])
        nc.scalar.copy(rb[:, g, :], ssum[:, 0, :])  # fp32 -> bf16 on ACT
        ev = esb[:, g, 0:9, :]
        nc.vector.tensor_tensor(
            ev, ev,
            rb[:, g, :].unsqueeze(1).broadcast_to([P, 9, T]),
            op=ALU.mult,
        )

        # enhanced[c,d,t] = sum_n p_n[c,t] * v_n[c,d,t] + blue  on PE PSUM
        for hf in range(2):
            dsl = slice(8 * hf, 8 * hf + 8)
            acc = psA.tile([P, 8 * T], F32, tag="psA")
            for n in range(9):
                di, dj = n // 3, n % 3
                vf = (v1, vun, v2)[dj][:, g, dsl, :]
                pe = esb[:, g, n, :].unsqueeze(1)
                tn = avp.tile([P, 8, T], BF16, tag="avprod")
                if di == 1:
                    nc.vector.tensor_tensor(
                        tn[:], vf, pe.broadcast_to([P, 8, T]), op=ALU.mult,
                    )
                elif di == 0:  # V row I-1: out tokens t<16 get zero
                    nc.vector.memset(tn[:, :, 0:16], 0.0)
                    nc.vector.tensor_tensor(
                        tn[:, :, 16:T], vf[:, :, 0:T - 16],
                        pe[:, :, 16:T].broadcast_to([P, 8, T - 16]),
                        op=ALU.mult,
                    )
                else:  # di == 2: out tokens t>=240 get zero
                    nc.vector.memset(tn[:, :, T - 16:T], 0.0)
                    nc.vector.tensor_tensor(
                        tn[:, :, 0:T - 16], vf[:, :, 16:T],
                        pe[:, :, 0:T - 16].broadcast_to([P, 8, T - 16]),
                        op=ALU.mult,
                    )
                tf = tn[:].rearrange("p d t -> p (d t)")
                for j in range(4):
                    imm(acc[:, j * 512:(j + 1) * 512],
                        tf[:, j * 512:(j + 1) * 512],
                        start=(n == 0), stop=False)
            for j in range(4):  # residual: accumulate blue into PSUM
                cols = slice(8 * hf * T + j * 512, 8 * hf * T + (j + 1) * 512)
                imm(acc[:, j * 512:(j + 1) * 512], xb[:, g, cols],
                    start=False, stop=True)
            of = outp.tile([P, 8 * T], BF16, tag="outf")
            nc.scalar.activation(of[:], acc[:], ACTF.Identity)
            nc.sync.dma_start(out_d[:, g, 8 * hf * T:(8 * hf + 8) * T], of[:])


def _build():
    global _BUILT
    if _BUILT is None:
        nc = bacc.Bacc(
            "TRN2", target_bir_lowering=False, debug=False, num_devices=N_CORES
        )
        with tile.TileContext(nc) as tc:
            with ExitStack() as ctx:
                _emit(ctx, tc)
        nc.compile()
        _BUILT = nc
    return _BUILT


def _tokenize(x: np.ndarray) -> np.ndarray:
    """[C,H,W] -> [C, D*T] d-major token order: index = (u,v,I,J)."""
    c = x.shape[0]
    return (
        x.reshape(c, NH, TS, NW, TS).transpose(0, 2, 4, 1, 3).reshape(c, PIX)
    )


def _untokenize(y: np.ndarray) -> np.ndarray:
    """[C, D*T] d-major token order -> [C, H, W]."""
    c = y.shape[0]
    return (
        y.reshape(c, TS, TS, NH, NW).transpose(0, 3, 1, 4, 2).reshape(c, H, W)
    )


def _part_fold(x: np.ndarray) -> np.ndarray:
    """[C, F] -> [P, C//P, F] partition-major fold."""
    return np.ascontiguousarray(
        x.reshape(C // P, P, -1).transpose(1, 0, 2)
    )


def _prep_maps(blue_feat, white_feat, Wq, bq, Wk, bk, Wv, bv):
    bf16 = ml_dtypes.bfloat16
    wall = np.stack([np.asarray(w, np.float32).T for w in (Wq, Wk, Wv)])  # [3,a,c]
    wall = np.ascontiguousarray(
        wall.reshape(3, 2, P, C).transpose(2, 0, 1, 3)
    ).reshape(P, 3 * G * C)  # [P, (proj, a_hi, c)]
    ball = np.ascontiguousarray(
        np.stack([bq, bk, bv]).astype(np.float32).reshape(3, G, P).transpose(2, 0, 1)
    ).reshape(P, 6)
    smalls = np.concatenate(
        [np.eye(P, dtype=np.float32), wall, ball], axis=1
    ).astype(bf16)  # [P, 1670]: ident | W^T | biases
    maps = []
    for b in range(B):
        xbm = _part_fold(_tokenize(np.asarray(blue_feat[b], np.float32))).astype(bf16)
        xwm = _part_fold(_tokenize(np.asarray(white_feat[b], np.float32))).astype(bf16)
        maps.append({"xb": xbm, "xw": xwm, "smalls": smalls})
    return maps


def _gather(results) -> np.ndarray:
    out = np.empty((B, C, H, W), np.float32)
    for b in range(B):
        y = results[b]["out"]  # [P, G, PIX] bf16
        y = np.asarray(y, np.float32).transpose(1, 0, 2).reshape(C, PIX)
        out[b] = _untokenize(y)
    return out


def _install_ntff_hook():
    """The agent image's antenv lacks axon_hooks; synthesize it so
    run_bass_kernel_spmd(trace=True) can drive NTFF profiling via the
    injected libaxon_pjrt.so C ABI (mirrors trn_agent_boot.trn_boot)."""
    import contextlib
    import ctypes
    import types

    if "antenv.axon_hooks" in sys.modules:
        return
    so_path = "/opt/axon/libaxon_pjrt.so"
    lib = ctypes.CDLL(so_path)
    if not hasattr(lib, "axon_start_nrt_profile"):
        return
    lib.axon_start_nrt_profile.argtypes = [
        ctypes.POINTER(ctypes.c_int64),
        ctypes.c_size_t,
    ]
    lib.axon_start_nrt_profile.restype = ctypes.c_int64
    lib.axon_stop_nrt_profile.argtypes = [ctypes.c_char_p]
    lib.axon_stop_nrt_profile.restype = ctypes.c_int64

    @contextlib.contextmanager
    def _hook(output_dir, device_ids):
        import jax

        jax.devices()
        if device_ids:
            ids = (ctypes.c_int64 * len(device_ids))(*device_ids)
            rc = lib.axon_start_nrt_profile(ids, len(device_ids))
        else:
            rc = lib.axon_start_nrt_profile(None, 0)
        if rc != 0:
            raise RuntimeError(f"axon_start_nrt_profile rc={rc}")
        try:
            yield
        finally:
            n = lib.axon_stop_nrt_profile(str(output_dir).encode())
            print(f"ntff profile: {n} file(s) written to {output_dir}")

    mod = types.ModuleType("antenv.axon_hooks")
    mod.get_axon_ntff_profile_hook = lambda: _hook  # type: ignore[attr-defined]
    mod.set_axon_ntff_profile_hook = lambda h: None  # type: ignore[attr-defined]
    sys.modules["antenv.axon_hooks"] = mod


def run(trace=False, **inputs):
    nc = _build()
    maps = _prep_maps(**inputs)
    if trace:
        _install_ntff_hook()
    res = run_bass_kernel_spmd(nc, maps, list(range(N_CORES)), trace=trace)
    return _gather(res.results), res


def kernel(**inputs) -> np.ndarray:
    out, _ = run(trace=False, **inputs)
    return out


# revision 33
# speedup vs baseline: 1.0273x; 1.0273x over previous
"""Trainium2 Bass kernel for nn_CrossModalAttention (sparse per-channel 3x3
token-window attention).

Contract: kernel(**inputs) takes the FULL fp32 inputs (B=8,C=256,H=W=64) and
returns the FULL fp32 output.  Internally: data-parallel over batch across the
8 NeuronCores (1 batch element per core), params replicated.

Layout: everything on-chip is d-major — activations are [c, d, t] where d is
the pixel-within-token index (16) and t = (I,J) the token index (256).  This
makes every DVE elementwise op innermost-contiguous over t (bf16 2x mode),
makes the per-d / per-n PE reduction matmuls read contiguous columns, and
turns the attention-weight broadcast over d into a free outer-dim 0-stride.

v2 schedule (vs v1):
  - inputs DMA'd in column chunks; Q/K projections interleaved per u-pair so
    the DVE QK products start as soon as the first half of Q/K lands.
  - DVE program order: QKprod(g0), QKprod(g1), softmax(g0), AVprod(g0),
    softmax(g1), AVprod(g1) — the DVE never waits on the PE mid-stream.
  - unpadded V copy (vun) gives all 9 AV products DVE 2x mode (v1: 6 of 9).
  - softmax: pairwise tree partly in bf16, reciprocal_approx_fast, recip
    cast to bf16 on the scalar engine.
  - residual add folded into the PE's AV accumulation (identity-weight matmul
    of blue into PSUM); ACT evacuates straight to bf16.
  - output stored as bf16 (host casts back to f32) — halves the store tail.
"""

import os
import sys
from contextlib import ExitStack

import numpy as np

for _p in ("/opt/trn_rl_repo",):
    if _p not in sys.path and os.path.isdir(_p):
        sys.path.insert(0, _p)

import ml_dtypes  # noqa: E402

import concourse.bacc as bacc  # noqa: E402
import concourse.bass as bass  # noqa: E402
import concourse.tile as tile  # noqa: E402
from concourse import mybir  # noqa: E402
from concourse.bass_utils import run_bass_kernel_spmd  # noqa: E402

BF16 = mybir.dt.bfloat16
F32 = mybir.dt.float32
ALU = mybir.AluOpType
ACTF = mybir.ActivationFunctionType

B, C, H, W = 8, 256, 64, 64
TS = 4                      # token size
NH = H // TS                # 16 token rows
NW = W // TS                # 16 token cols
T = NH * NW                 # 256 tokens
D = TS * TS                 # 16 pixels per token
G = 2                       # channel groups of 128
P = 128
PIX = H * W                 # 4096
GRID = NH + 2               # 18 (zero-padded token grid)
SCALE = float(D) ** -0.5    # 0.25
N_CORES = 8
CH = 1024                   # proj psum chunk: 4 d-planes x 256 t

_BUILT = None


def _emit(ctx: ExitStack, tc: "tile.TileContext"):
    nc = tc.nc

    # ---- DRAM I/O (per-core shard); activations d-major [p, g, d, t] ----
    xb_d = nc.dram_tensor("xb", [P, G, PIX], BF16, kind="ExternalInput").ap()
    xw_d = nc.dram_tensor("xw", [P, G, PIX], BF16, kind="ExternalInput").ap()
    sm_d = nc.dram_tensor("smalls", [P, 1670], BF16, kind="ExternalInput").ap()
    out_d = nc.dram_tensor("out", [P, G, PIX], BF16, kind="ExternalOutput").ap()

    consts = ctx.enter_context(tc.tile_pool(name="consts", bufs=1))

    # ---- persistent SBUF tiles ----
    xb = consts.tile([P, G, PIX], BF16)          # blue, d-major token order
    sm = consts.tile([P, 1670], BF16)            # packed: ident | W^T | biases
    ident = sm[:, 0:P]
    wall = sm[:, P:P + 3 * G * C].rearrange("p (w g c) -> p w g c", w=3, g=G)
    ball = sm[:, P + 3 * G * C:]                 # bf16 biases, col = proj*2+g
    xwpool = tc.tile_pool(name="xwp", bufs=1)
    xw = xwpool.__enter__().tile([P, G, PIX], BF16, tag="xw")  # proj phase only
    qsb = consts.tile([P, G, D, T], BF16)
    # K/V each as three flat copies in ONE tile, slot dj: 0 = shift+1
    # (kall[0][t]=K[t-1], the J-1 view), 1 = center, 2 = shift-1 (J+1 view).
    # Shifts built from the center by one contiguous SBUF->SBUF DMA; J-edge
    # columns re-zeroed by small memsets.  All neighbor views are then
    # even-offset flat slices -> DVE 2x mode, and the three dj slots of one
    # di row are a single strided AP -> one fused product TT per (di, dh).
    kall = consts.tile([P, G, 3, D, T], BF16)
    vall = consts.tile([P, G, 3, D, T], BF16)
    esb = consts.tile([P, G, 9, T], BF16)        # exp(logits), slot-major
    ballf = consts.tile([P, 6], F32)             # fp32 biases (DVE evac path)
    tA = consts.tile([P, 4, T], BF16)            # softmax tree scratch
    tB = consts.tile([P, 2, T], F32)
    ssum = consts.tile([P, 2, T], F32)           # [0]=sum, [1]=recip (per g reuse)
    rb = consts.tile([P, G, T], BF16)            # recip, bf16 for 2x scale

    # input DMA, column-chunked so projections can start on chunk 0
    nc.sync.dma_start(sm[:], sm_d[:])
    for c in range(PIX // CH):
        cs = slice(c * CH, (c + 1) * CH)
        nc.sync.dma_start(xb[:, :, cs], xb_d[:, :, cs])
        nc.sync.dma_start(xw[:, :, cs], xw_d[:, :, cs])

    nc.scalar.copy(ballf[:], ball[:])  # bf16 -> fp32 biases for DVE evacs

    # ---- projections: out[c, d, t] = sum_a W[c,a] x[a, d, t] + b[c] ----
    with tc.tile_pool(name="psumP", bufs=2, space="PSUM") as psum:
        # emission plan: Q/K interleaved per u-pair (QK products can start
        # after u0,u1 of Q+K for a g), V projections last.
        plan = []
        for g in range(G):
            for up in range(2):
                for proj in (0, 1):
                    plan.append((proj, g, 2 * up))
                    plan.append((proj, g, 2 * up + 1))
        for g in range(G):
            for u in range(4):
                plan.append((2, g, u))

        for proj, g, u in plan:
            src = xb if proj == 0 else xw
            bias_ap = ball[:, proj * 2 + g: proj * 2 + g + 1]
            pt = psum.tile([P, CH], F32, tag="psP")
            for h in range(2):  # h-outer: one weight load per half
                for j in range(CH // 512):
                    cols = slice(u * CH + j * 512, u * CH + (j + 1) * 512)
                    mm = nc.tensor.matmul(
                        pt[:, j * 512:(j + 1) * 512],
                        wall[:, proj, h, g * P:(g + 1) * P],
                        src[:, h, cols],
                        start=(h == 0),
                        stop=(h == 1),
                    )
                    if j > 0:
                        mm.ins.ldweights = False
            ds = slice(4 * u, 4 * u + 4)
            if proj == 0 and g == 0:  # Q g0 evac on DVE (idle at startup)
                nc.vector.tensor_scalar_add(
                    qsb[:, g, ds, :], pt[:], ballf[:, g:g + 1]
                )
            elif proj == 0:
                nc.scalar.activation(
                    qsb[:, g, ds, :], pt[:], ACTF.Identity, bias=bias_ap
                )
            else:
                tall = kall if proj == 1 else vall
                nc.scalar.activation(
                    tall[:, g, 1, ds, :], pt[:], ACTF.Identity, bias=bias_ap
                )
                if u % 2 == 1:  # shifted copies per dh half (2048 elems)
                    dh = u // 2
                    a, b = dh * 2048, dh * 2048 + 2048
                    uf = tall[:, g, 1].rearrange("p d t -> p (d t)")
                    s1f = tall[:, g, 0].rearrange("p d t -> p (d t)")
                    s2f = tall[:, g, 2].rearrange("p d t -> p (d t)")
                    nc.sync.dma_start(
                        s1f[:, max(a, 1):b], uf[:, max(a, 1) - 1:b - 1]
                    )
                    nc.sync.dma_start(
                        s2f[:, a:min(b, PIX - 1)], uf[:, a + 1:min(b, PIX - 1) + 1]
                    )

    xwpool.__exit__(None, None, None)  # free xw; attention pools reuse it
    prod = ctx.enter_context(tc.tile_pool(name="prod", bufs=3))
    avp = ctx.enter_context(tc.tile_pool(name="avp", bufs=3))
    outp = ctx.enter_context(tc.tile_pool(name="outf", bufs=2))

    # ---- attention ----
    # neighbor n = 3*di + dj, di,dj in {0,1,2}.  dj selects the flat-copy
    # slot of kall/vall; di is a +-16 token shift with product-edge memsets.
    def jzero(tall, g):
        """Zero the J-edge columns of the shifted slots (after their DMAs)."""
        for dh in range(2):
            dsl = slice(8 * dh, 8 * dh + 8)
            nc.vector.memset(tall[:, g, 0, dsl, 0:T:NW], 0.0)
            nc.vector.memset(tall[:, g, 2, dsl, NW - 1:T:NW], 0.0)

    first_ident = [True]

    def imm(out_ap, mov_ap, start, stop):
        m = nc.tensor.matmul(out_ap, ident, mov_ap, start=start, stop=stop)
        if first_ident[0]:
            first_ident[0] = False
        else:
            m.ins.ldweights = False
        return m

    # --- QK phase: products (DVE), d-reduction (PE), exp (ACT) ---
    with tc.tile_pool(name="psumL", bufs=2, space="PSUM") as psL:
        for g in range(G):
            jzero(kall, g)
            qv = qsb[:, g]  # [P, D, T]
            for di in range(3):          # group = token row di, slots 3di+dj
                halves = []
                for dh in range(2):
                    ph = prod.tile([P, 3, D // 2, T], BF16, tag="prod")
                    dsl = slice(8 * dh, 8 * dh + 8)
                    kf = kall[:, g, :, dsl, :]  # [P, 3, 8, T]
                    if di == 1:
                        nc.vector.tensor_tensor(
                            ph[:],
                            qv[:, dsl, :].unsqueeze(1).broadcast_to(
                                [P, 3, 8, T]),
                            kf, op=ALU.mult,
                        )
                    elif di == 0:  # K row I-1: valid t >= 16
                        nc.vector.memset(ph[:, :, :, 0:16], 0.0)
                        nc.vector.tensor_tensor(
                            ph[:, :, :, 16:T],
                            qv[:, dsl, 16:T].unsqueeze(1).broadcast_to(
                                [P, 3, 8, T - 16]),
                            kf[:, :, :, 0:T - 16], op=ALU.mult,
                        )
                    else:  # di == 2: K row I+1: valid t < 240
                        nc.vector.memset(ph[:, :, :, T - 16:T], 0.0)
                        nc.vector.tensor_tensor(
                            ph[:, :, :, 0:T - 16],
                            qv[:, dsl, 0:T - 16].unsqueeze(1).broadcast_to(
                                [P, 3, 8, T - 16]),
                            kf[:, :, :, 16:T], op=ALU.mult,
                        )
                    halves.append(ph)
                lp = psL.tile([P, 768], F32, tag="psL")
                for d in range(D):  # accumulate over d on PE (identity weights)
                    imm(lp[:, 0:512], halves[d // 8][:, 0:2, d % 8, :],
                        start=(d == 0), stop=(d == D - 1))
                    imm(lp[:, 512:768], halves[d // 8][:, 2:3, d % 8, :],
                        start=(d == 0), stop=(d == D - 1))
                nc.scalar.activation(  # e = exp(scale*l) -> [slot, t]
                    esb[:, g, 3 * di:3 * di + 3, :],
                    lp[:], ACTF.Exp, scale=SCALE,
                )

    # --- softmax normalize + AV phase ---
    psA = ctx.enter_context(tc.tile_pool(name="psumA", bufs=2, space="PSUM"))
    for g in range(G):
        jzero(vall, g)
        # denominator: pairwise tree (bf16 first level, then fp32)
        nc.vector.tensor_tensor(
            tA[:], esb[:, g, 0:4, :], esb[:, g, 4:8, :], op=ALU.add
        )
        nc.vector.tensor_tensor(tB[:], tA[:, 0:2, :], tA[:, 2:4, :], op=ALU.add)
        nc.vector.tensor_tensor(
            ssum[:, 0, :], tB[:, 0, :], tB[:, 1, :], op=ALU.add
        )
        nc.vector.tensor_tensor(
            ssum[:, 1, :], ssum[:, 0, :], esb[:, g, 8, :], op=ALU.add
        )
        nc.vector.reciprocal_approx_fast(ssum[:, 0, :], ssum[:, 1, :])
        nc.scalar.copy(rb[:, g, :], ssum[:, 0, :])  # fp32 -> bf16 on ACT
        ev = esb[:, g, 0:9, :]
        nc.vector.tensor_tensor(
            ev, ev,
            rb[:, g, :].unsqueeze(1).broadcast_to([P, 9, T]),
            op=ALU.mult,
        )

        # enhanced[c,d,t] = sum_n p_n[c,t] * v_n[c,d,t] + blue  on PE PSUM
        for qq in range(4):  # d-quarters of 4 planes
            dsl = slice(4 * qq, 4 * qq + 4)
            acc = psA.tile([P, 4 * T], F32, tag="psA")
            for di in range(3):
                vf = vall[:, g, :, dsl, :]  # [P, 3, 4, T]
                pe = esb[:, g, 3 * di:3 * di + 3, :].unsqueeze(2)
                tn = avp.tile([P, 3, 4, T], BF16, tag="avprod")
                if di == 1:
                    nc.vector.tensor_tensor(
                        tn[:], vf, pe.broadcast_to([P, 3, 4, T]), op=ALU.mult,
                    )
                elif di == 0:  # V row I-1: out tokens t<16 get zero
                    nc.vector.memset(tn[:, :, :, 0:16], 0.0)
                    nc.vector.tensor_tensor(
                        tn[:, :, :, 16:T], vf[:, :, :, 0:T - 16],
                        pe[:, :, :, 16:T].broadcast_to([P, 3, 4, T - 16]),
                        op=ALU.mult,
                    )
                else:  # di == 2: out tokens t>=240 get zero
                    nc.vector.memset(tn[:, :, :, T - 16:T], 0.0)
                    nc.vector.tensor_tensor(
                        tn[:, :, :, 0:T - 16], vf[:, :, :, 16:T],
                        pe[:, :, :, 0:T - 16].broadcast_to([P, 3, 4, T - 16]),
                        op=ALU.mult,
                    )
                tf = tn[:].rearrange("p w d t -> p (w d t)")
                for w in range(3):
                    for j in range(2):
                        imm(acc[:, j * 512:(j + 1) * 512],
                            tf[:, w * 1024 + j * 512:w * 1024 + (j + 1) * 512],
                            start=(di == 0 and w == 0), stop=False)
            for j in range(2):  # residual: accumulate blue into PSUM
                cols = slice(4 * qq * T + j * 512, 4 * qq * T + (j + 1) * 512)
                imm(acc[:, j * 512:(j + 1) * 512], xb[:, g, cols],
                    start=False, stop=True)
            of = outp.tile([P, 4 * T], BF16, tag="outf")
            nc.scalar.activation(of[:], acc[:], ACTF.Identity)
            nc.sync.dma_start(out_d[:, g, 4 * qq * T:(4 * qq + 4) * T], of[:])


def _build():
    global _BUILT
    if _BUILT is None:
        nc = bacc.Bacc(
            "TRN2", target_bir_lowering=False, debug=False, num_devices=N_CORES
        )
        with tile.TileContext(nc) as tc:
            with ExitStack() as ctx:
                _emit(ctx, tc)
        nc.compile()
        _BUILT = nc
    return _BUILT


def _tokenize(x: np.ndarray) -> np.ndarray:
    """[C,H,W] -> [C, D*T] d-major token order: index = (u,v,I,J)."""
    c = x.shape[0]
    return (
        x.reshape(c, NH, TS, NW, TS).transpose(0, 2, 4, 1, 3).reshape(c, PIX)
    )


def _untokenize(y: np.ndarray) -> np.ndarray:
    """[C, D*T] d-major token order -> [C, H, W]."""
    c = y.shape[0]
    return (
        y.reshape(c, TS, TS, NH, NW).transpose(0, 3, 1, 4, 2).reshape(c, H, W)
    )


def _part_fold(x: np.ndarray) -> np.ndarray:
    """[C, F] -> [P, C//P, F] partition-major fold."""
    return np.ascontiguousarray(
        x.reshape(C // P, P, -1).transpose(1, 0, 2)
    )


def _prep_maps(blue_feat, white_feat, Wq, bq, Wk, bk, Wv, bv):
    bf16 = ml_dtypes.bfloat16
    wall = np.stack([np.asarray(w, np.float32).T for w in (Wq, Wk, Wv)])  # [3,a,c]
    wall = np.ascontiguousarray(
        wall.reshape(3, 2, P, C).transpose(2, 0, 1, 3)
    ).reshape(P, 3 * G * C)  # [P, (proj, a_hi, c)]
    ball = np.ascontiguousarray(
        np.stack([bq, bk, bv]).astype(np.float32).reshape(3, G, P).transpose(2, 0, 1)
    ).reshape(P, 6)
    smalls = np.concatenate(
        [np.eye(P, dtype=np.float32), wall, ball], axis=1
    ).astype(bf16)  # [P, 1670]: ident | W^T | biases
    maps = []
    for b in range(B):
        xbm = _part_fold(_tokenize(np.asarray(blue_feat[b], np.float32))).astype(bf16)
        xwm = _part_fold(_tokenize(np.asarray(white_feat[b], np.float32))).astype(bf16)
        maps.append({"xb": xbm, "xw": xwm, "smalls": smalls})
    return maps


def _gather(results) -> np.ndarray:
    out = np.empty((B, C, H, W), np.float32)
    for b in range(B):
        y = results[b]["out"]  # [P, G, PIX] bf16
        y = np.asarray(y, np.float32).transpose(1, 0, 2).reshape(C, PIX)
        out[b] = _untokenize(y)
    return out


def _install_ntff_hook():
    """The agent image's antenv lacks axon_hooks; synthesize it so
    run_bass_kernel_spmd(trace=True) can drive NTFF profiling via the
    injected libaxon_pjrt.so C ABI (mirrors trn_agent_boot.trn_boot)."""
    import contextlib
    import ctypes
    import types

    if "antenv.axon_hooks" in sys.modules:
        return
    so_path = "/opt/axon/libaxon_pjrt.so"
    lib = ctypes.CDLL(so_path)
    if not hasattr(lib, "axon_start_nrt_profile"):
        return
    lib.axon_start_nrt_profile.argtypes = [
        ctypes.POINTER(ctypes.c_int64),
        ctypes.c_size_t,
    ]
    lib.axon_start_nrt_profile.restype = ctypes.c_int64
    lib.axon_stop_nrt_profile.argtypes = [ctypes.c_char_p]
    lib.axon_stop_nrt_profile.restype = ctypes.c_int64

    @contextlib.contextmanager
    def _hook(output_dir, device_ids):
        import jax

        jax.devices()
        if device_ids:
            ids = (ctypes.c_int64 * len(device_ids))(*device_ids)
            rc = lib.axon_start_nrt_profile(ids, len(device_ids))
        else:
            rc = lib.axon_start_nrt_profile(None, 0)
        if rc != 0:
            raise RuntimeError(f"axon_start_nrt_profile rc={rc}")
        try:
            yield
        finally:
            n = lib.axon_stop_nrt_profile(str(output_dir).encode())
            print(f"ntff profile: {n} file(s) written to {output_dir}")

    mod = types.ModuleType("antenv.axon_hooks")
    mod.get_axon_ntff_profile_hook = lambda: _hook  # type: ignore[attr-defined]
    mod.set_axon_ntff_profile_hook = lambda h: None  # type: ignore[attr-defined]
    sys.modules["antenv.axon_hooks"] = mod


def run(trace=False, **inputs):
    nc = _build()
    maps = _prep_maps(**inputs)
    if trace:
        _install_ntff_hook()
    res = run_bass_kernel_spmd(nc, maps, list(range(N_CORES)), trace=trace)
    return _gather(res.results), res


def kernel(**inputs) -> np.ndarray:
    out, _ = run(trace=False, **inputs)
    return out


# revision 35
# speedup vs baseline: 1.0701x; 1.0416x over previous
"""Trainium2 Bass kernel for nn_CrossModalAttention (sparse per-channel 3x3
token-window attention).

Contract: kernel(**inputs) takes the FULL fp32 inputs (B=8,C=256,H=W=64) and
returns the FULL fp32 output.  Internally: data-parallel over batch across the
8 NeuronCores (1 batch element per core), params replicated.

Layout: everything on-chip is d-major — activations are [c, d, t] where d is
the pixel-within-token index (16) and t = (I,J) the token index (256).  This
makes every DVE elementwise op innermost-contiguous over t (bf16 2x mode),
makes the per-d / per-n PE reduction matmuls read contiguous columns, and
turns the attention-weight broadcast over d into a free outer-dim 0-stride.

v2 schedule (vs v1):
  - inputs DMA'd in column chunks; Q/K projections interleaved per u-pair so
    the DVE QK products start as soon as the first half of Q/K lands.
  - DVE program order: QKprod(g0), QKprod(g1), softmax(g0), AVprod(g0),
    softmax(g1), AVprod(g1) — the DVE never waits on the PE mid-stream.
  - unpadded V copy (vun) gives all 9 AV products DVE 2x mode (v1: 6 of 9).
  - softmax: pairwise tree partly in bf16, reciprocal_approx_fast, recip
    cast to bf16 on the scalar engine.
  - residual add folded into the PE's AV accumulation (identity-weight matmul
    of blue into PSUM); ACT evacuates straight to bf16.
  - output stored as bf16 (host casts back to f32) — halves the store tail.
"""

import os
import sys
from contextlib import ExitStack

import numpy as np

for _p in ("/opt/trn_rl_repo",):
    if _p not in sys.path and os.path.isdir(_p):
        sys.path.insert(0, _p)

import ml_dtypes  # noqa: E402

import concourse.bacc as bacc  # noqa: E402
import concourse.bass as bass  # noqa: E402
import concourse.tile as tile  # noqa: E402
from concourse import mybir  # noqa: E402
from concourse.bass_utils import run_bass_kernel_spmd  # noqa: E402

BF16 = mybir.dt.bfloat16
F32 = mybir.dt.float32
ALU = mybir.AluOpType
ACTF = mybir.ActivationFunctionType

B, C, H, W = 8, 256, 64, 64
TS = 4                      # token size
NH = H // TS                # 16 token rows
NW = W // TS                # 16 token cols
T = NH * NW                 # 256 tokens
D = TS * TS                 # 16 pixels per token
G = 2                       # channel groups of 128
P = 128
PIX = H * W                 # 4096
GRID = NH + 2               # 18 (zero-padded token grid)
SCALE = float(D) ** -0.5    # 0.25
N_CORES = 8
CH = 1024                   # proj psum chunk: 4 d-planes x 256 t

_BUILT = None


def _emit(ctx: ExitStack, tc: "tile.TileContext"):
    nc = tc.nc

    # ---- DRAM I/O (per-core shard); activations d-major [p, g, d, t] ----
    xb_d = nc.dram_tensor("xb", [P, G, PIX], BF16, kind="ExternalInput").ap()
    xw_d = nc.dram_tensor("xw", [P, G, PIX], BF16, kind="ExternalInput").ap()
    sm_d = nc.dram_tensor("smalls", [P, 1670], BF16, kind="ExternalInput").ap()
    out_d = nc.dram_tensor("out", [P, G, PIX], BF16, kind="ExternalOutput").ap()

    consts = ctx.enter_context(tc.tile_pool(name="consts", bufs=1))

    # ---- persistent SBUF tiles ----
    xb = consts.tile([P, G, PIX], BF16)          # blue, d-major token order
    sm = consts.tile([P, 1670], BF16)            # packed: ident | W^T | biases
    ident = sm[:, 0:P]
    wall = sm[:, P:P + 3 * G * C].rearrange("p (w g c) -> p w g c", w=3, g=G)
    ball = sm[:, P + 3 * G * C:]                 # bf16 biases, col = proj*2+g
    xwpool = tc.tile_pool(name="xwp", bufs=1)
    xw = xwpool.__enter__().tile([P, G, PIX], BF16, tag="xw")  # proj phase only
    qsb = consts.tile([P, G, D, T], BF16)
    # K/V each as three flat copies in ONE tile, slot dj: 0 = shift+1
    # (kall[0][t]=K[t-1], the J-1 view), 1 = center, 2 = shift-1 (J+1 view).
    # Shifts built from the center by one contiguous SBUF->SBUF DMA; J-edge
    # columns re-zeroed by small memsets.  All neighbor views are then
    # even-offset flat slices -> DVE 2x mode, and the three dj slots of one
    # di row are a single strided AP -> one fused product TT per (di, dh).
    kall = consts.tile([P, G, 3, D, T], BF16)
    vall = consts.tile([P, G, 3, D, T], BF16)
    esb = consts.tile([P, G, 9, T], BF16)        # exp(logits), slot-major
    ballf = consts.tile([P, 6], F32)             # fp32 biases (DVE evac path)
    tA = consts.tile([P, 4, T], BF16)            # softmax tree scratch
    tB = consts.tile([P, 2, T], F32)
    ssum = consts.tile([P, 2, T], F32)           # [0]=sum, [1]=recip (per g reuse)
    rb = consts.tile([P, G, T], BF16)            # recip, bf16 for 2x scale

    # input DMA, column-chunked so projections can start on chunk 0
    nc.sync.dma_start(sm[:], sm_d[:])
    for c in range(PIX // CH):
        cs = slice(c * CH, (c + 1) * CH)
        nc.sync.dma_start(xb[:, :, cs], xb_d[:, :, cs])
        nc.sync.dma_start(xw[:, :, cs], xw_d[:, :, cs])

    nc.scalar.copy(ballf[:], ball[:])  # bf16 -> fp32 biases for DVE evacs

    # ---- projections: out[c, d, t] = sum_a W[c,a] x[a, d, t] + b[c] ----
    with tc.tile_pool(name="psumP", bufs=3, space="PSUM") as psum:
        # emission plan: Q/K interleaved per u-pair (QK products can start
        # after u0,u1 of Q+K for a g), V projections last.
        plan = []
        for g in range(G):
            for up in range(2):
                for proj in (0, 1):
                    plan.append((proj, g, 2 * up))
                    plan.append((proj, g, 2 * up + 1))
        for g in range(G):
            for u in range(4):
                plan.append((2, g, u))

        for proj, g, u in plan:
            src = xb if proj == 0 else xw
            bias_ap = ball[:, proj * 2 + g: proj * 2 + g + 1]
            pt = psum.tile([P, CH], F32, tag="psP")
            for h in range(2):  # h-outer: one weight load per half
                for j in range(CH // 512):
                    cols = slice(u * CH + j * 512, u * CH + (j + 1) * 512)
                    mm = nc.tensor.matmul(
                        pt[:, j * 512:(j + 1) * 512],
                        wall[:, proj, h, g * P:(g + 1) * P],
                        src[:, h, cols],
                        start=(h == 0),
                        stop=(h == 1),
                    )
                    if j > 0:
                        mm.ins.ldweights = False
            ds = slice(4 * u, 4 * u + 4)
            if proj == 0:  # Q evacs on DVE, K/V on ACT (parallel drains)
                nc.vector.tensor_scalar_add(
                    qsb[:, g, ds, :], pt[:], ballf[:, g:g + 1]
                )
            else:
                tall = kall if proj == 1 else vall
                nc.scalar.activation(
                    tall[:, g, 1, ds, :], pt[:], ACTF.Identity, bias=bias_ap
                )
                if u % 2 == 1:  # shifted copies per dh half (2048 elems)
                    dh = u // 2
                    a, b = dh * 2048, dh * 2048 + 2048
                    uf = tall[:, g, 1].rearrange("p d t -> p (d t)")
                    s1f = tall[:, g, 0].rearrange("p d t -> p (d t)")
                    s2f = tall[:, g, 2].rearrange("p d t -> p (d t)")
                    nc.sync.dma_start(
                        s1f[:, max(a, 1):b], uf[:, max(a, 1) - 1:b - 1]
                    )
                    nc.sync.dma_start(
                        s2f[:, a:min(b, PIX - 1)], uf[:, a + 1:min(b, PIX - 1) + 1]
                    )

    xwpool.__exit__(None, None, None)  # free xw; attention pools reuse it
    prod = ctx.enter_context(tc.tile_pool(name="prod", bufs=3))
    avp = ctx.enter_context(tc.tile_pool(name="avp", bufs=3))
    outp = ctx.enter_context(tc.tile_pool(name="outf", bufs=2))

    # ---- attention ----
    # neighbor n = 3*di + dj, di,dj in {0,1,2}.  dj selects the flat-copy
    # slot of kall/vall; di is a +-16 token shift with product-edge memsets.
    def jzero(tall, g):
        """Zero the J-edge columns of the shifted slots (after their DMAs)."""
        for dh in range(2):
            dsl = slice(8 * dh, 8 * dh + 8)
            nc.vector.memset(tall[:, g, 0, dsl, 0:T:NW], 0.0)
            nc.vector.memset(tall[:, g, 2, dsl, NW - 1:T:NW], 0.0)

    first_ident = [True]

    def imm(out_ap, mov_ap, start, stop):
        m = nc.tensor.matmul(out_ap, ident, mov_ap, start=start, stop=stop)
        if first_ident[0]:
            first_ident[0] = False
        else:
            m.ins.ldweights = False
        return m

    # --- QK phase: products (DVE), d-reduction (PE), exp (ACT) ---
    with tc.tile_pool(name="psumL", bufs=2, space="PSUM") as psL:
        for g in range(G):
            jzero(kall, g)
            qv = qsb[:, g]  # [P, D, T]
            for di in range(3):          # group = token row di, slots 3di+dj
                halves = []
                for dh in range(2):
                    ph = prod.tile([P, 3, D // 2, T], BF16, tag="prod")
                    dsl = slice(8 * dh, 8 * dh + 8)
                    kf = kall[:, g, :, dsl, :]  # [P, 3, 8, T]
                    if di == 1:
                        nc.vector.tensor_tensor(
                            ph[:],
                            qv[:, dsl, :].unsqueeze(1).broadcast_to(
                                [P, 3, 8, T]),
                            kf, op=ALU.mult,
                        )
                    elif di == 0:  # K row I-1: valid t >= 16
                        nc.vector.memset(ph[:, :, :, 0:16], 0.0)
                        nc.vector.tensor_tensor(
                            ph[:, :, :, 16:T],
                            qv[:, dsl, 16:T].unsqueeze(1).broadcast_to(
                                [P, 3, 8, T - 16]),
                            kf[:, :, :, 0:T - 16], op=ALU.mult,
                        )
                    else:  # di == 2: K row I+1: valid t < 240
                        nc.vector.memset(ph[:, :, :, T - 16:T], 0.0)
                        nc.vector.tensor_tensor(
                            ph[:, :, :, 0:T - 16],
                            qv[:, dsl, 0:T - 16].unsqueeze(1).broadcast_to(
                                [P, 3, 8, T - 16]),
                            kf[:, :, :, 16:T], op=ALU.mult,
                        )
                    halves.append(ph)
                lp = psL.tile([P, 768], F32, tag="psL")
                for d in range(D):  # accumulate over d on PE (identity weights)
                    imm(lp[:, 0:512], halves[d // 8][:, 0:2, d % 8, :],
                        start=(d == 0), stop=(d == D - 1))
                    imm(lp[:, 512:768], halves[d // 8][:, 2:3, d % 8, :],
                        start=(d == 0), stop=(d == D - 1))
                nc.scalar.activation(  # e = exp(scale*l) -> [slot, t]
                    esb[:, g, 3 * di:3 * di + 3, :],
                    lp[:], ACTF.Exp, scale=SCALE,
                )

    # --- softmax normalize + AV phase ---
    psA = ctx.enter_context(tc.tile_pool(name="psumA", bufs=2, space="PSUM"))
    for g in range(G):
        jzero(vall, g)
        # denominator: pairwise tree (bf16 first level, then fp32)
        nc.vector.tensor_tensor(
            tA[:], esb[:, g, 0:4, :], esb[:, g, 4:8, :], op=ALU.add
        )
        nc.vector.tensor_tensor(tB[:], tA[:, 0:2, :], tA[:, 2:4, :], op=ALU.add)
        nc.vector.tensor_tensor(
            ssum[:, 0, :], tB[:, 0, :], tB[:, 1, :], op=ALU.add
        )
        nc.vector.tensor_tensor(
            ssum[:, 1, :], ssum[:, 0, :], esb[:, g, 8, :], op=ALU.add
        )
        nc.vector.reciprocal_approx_fast(ssum[:, 0, :], ssum[:, 1, :])
        nc.scalar.copy(rb[:, g, :], ssum[:, 0, :])  # fp32 -> bf16 on ACT
        ev = esb[:, g, 0:9, :]
        nc.vector.tensor_tensor(
            ev, ev,
            rb[:, g, :].unsqueeze(1).broadcast_to([P, 9, T]),
            op=ALU.mult,
        )

        # enhanced[c,d,t] = sum_n p_n[c,t] * v_n[c,d,t] + blue  on PE PSUM
        for qq in range(4):  # d-quarters of 4 planes
            dsl = slice(4 * qq, 4 * qq + 4)
            acc = psA.tile([P, 4 * T], F32, tag="psA")
            for di in range(3):
                vf = vall[:, g, :, dsl, :]  # [P, 3, 4, T]
                pe = esb[:, g, 3 * di:3 * di + 3, :].unsqueeze(2)
                tn = avp.tile([P, 3, 4, T], BF16, tag="avprod")
                if di == 1:
                    nc.vector.tensor_tensor(
                        tn[:], vf, pe.broadcast_to([P, 3, 4, T]), op=ALU.mult,
                    )
                elif di == 0:  # V row I-1: out tokens t<16 get zero
                    nc.vector.memset(tn[:, :, :, 0:16], 0.0)
                    nc.vector.tensor_tensor(
                        tn[:, :, :, 16:T], vf[:, :, :, 0:T - 16],
                        pe[:, :, :, 16:T].broadcast_to([P, 3, 4, T - 16]),
                        op=ALU.mult,
                    )
                else:  # di == 2: out tokens t>=240 get zero
                    nc.vector.memset(tn[:, :, :, T - 16:T], 0.0)
                    nc.vector.tensor_tensor(
                        tn[:, :, :, 0:T - 16], vf[:, :, :, 16:T],
                        pe[:, :, :, 0:T - 16].broadcast_to([P, 3, 4, T - 16]),
                        op=ALU.mult,
                    )
                tf = tn[:].rearrange("p w d t -> p (w d t)")
                for w in range(3):
                    for j in range(2):
                        imm(acc[:, j * 512:(j + 1) * 512],
                            tf[:, w * 1024 + j * 512:w * 1024 + (j + 1) * 512],
                            start=(di == 0 and w == 0), stop=False)
            for j in range(2):  # residual: accumulate blue into PSUM
                cols = slice(4 * qq * T + j * 512, 4 * qq * T + (j + 1) * 512)
                imm(acc[:, j * 512:(j + 1) * 512], xb[:, g, cols],
                    start=False, stop=True)
            of = outp.tile([P, 4 * T], BF16, tag="outf")
            nc.scalar.activation(of[:], acc[:], ACTF.Identity)
            nc.sync.dma_start(out_d[:, g, 4 * qq * T:(4 * qq + 4) * T], of[:])


def _build():
    global _BUILT
    if _BUILT is None:
        nc = bacc.Bacc(
            "TRN2", target_bir_lowering=False, debug=False, num_devices=N_CORES
        )
        with tile.TileContext(nc) as tc:
            with ExitStack() as ctx:
                _emit(ctx, tc)
        nc.compile()
        _BUILT = nc
    return _BUILT


def _tokenize(x: np.ndarray) -> np.ndarray:
    """[C,H,W] -> [C, D*T] d-major token order: index = (u,v,I,J)."""
    c = x.shape[0]
    return (
        x.reshape(c, NH, TS, NW, TS).transpose(0, 2, 4, 1, 3).reshape(c, PIX)
    )


def _untokenize(y: np.ndarray) -> np.ndarray:
    """[C, D*T] d-major token order -> [C, H, W]."""
    c = y.shape[0]
    return (
        y.reshape(c, TS, TS, NH, NW).transpose(0, 3, 1, 4, 2).reshape(c, H, W)
    )


def _part_fold(x: np.ndarray) -> np.ndarray:
    """[C, F] -> [P, C//P, F] partition-major fold."""
    return np.ascontiguousarray(
        x.reshape(C // P, P, -1).transpose(1, 0, 2)
    )


def _prep_maps(blue_feat, white_feat, Wq, bq, Wk, bk, Wv, bv):
    bf16 = ml_dtypes.bfloat16
    wall = np.stack([np.asarray(w, np.float32).T for w in (Wq, Wk, Wv)])  # [3,a,c]
    wall = np.ascontiguousarray(
        wall.reshape(3, 2, P, C).transpose(2, 0, 1, 3)
    ).reshape(P, 3 * G * C)  # [P, (proj, a_hi, c)]
    ball = np.ascontiguousarray(
        np.stack([bq, bk, bv]).astype(np.float32).reshape(3, G, P).transpose(2, 0, 1)
    ).reshape(P, 6)
    smalls = np.concatenate(
        [np.eye(P, dtype=np.float32), wall, ball], axis=1
    ).astype(bf16)  # [P, 1670]: ident | W^T | biases
    maps = []
    for b in range(B):
        xbm = _part_fold(_tokenize(np.asarray(blue_feat[b], np.float32))).astype(bf16)
        xwm = _part_fold(_tokenize(np.asarray(white_feat[b], np.float32))).astype(bf16)
        maps.append({"xb": xbm, "xw": xwm, "smalls": smalls})
    return maps


def _gather(results) -> np.ndarray:
    out = np.empty((B, C, H, W), np.float32)
    for b in range(B):
        y = results[b]["out"]  # [P, G, PIX] bf16
        y = np.asarray(y, np.float32).transpose(1, 0, 2).reshape(C, PIX)
        out[b] = _untokenize(y)
    return out


def _install_ntff_hook():
    """The agent image's antenv lacks axon_hooks; synthesize it so
    run_bass_kernel_spmd(trace=True) can drive NTFF profiling via the
    injected libaxon_pjrt.so C ABI (mirrors trn_agent_boot.trn_boot)."""
    import contextlib
    import ctypes
    import types

    if "antenv.axon_hooks" in sys.modules:
        return
    so_path = "/opt/axon/libaxon_pjrt.so"
    lib = ctypes.CDLL(so_path)
    if not hasattr(lib, "axon_start_nrt_profile"):
        return
    lib.axon_start_nrt_profile.argtypes = [
        ctypes.POINTER(ctypes.c_int64),
        ctypes.c_size_t,
    ]
    lib.axon_start_nrt_profile.restype = ctypes.c_int64
    lib.axon_stop_nrt_profile.argtypes = [ctypes.c_char_p]
    lib.axon_stop_nrt_profile.restype = ctypes.c_int64

    @contextlib.contextmanager
    def _hook(output_dir, device_ids):
        import jax

        jax.devices()
        if device_ids:
            ids = (ctypes.c_int64 * len(device_ids))(*device_ids)
            rc = lib.axon_start_nrt_profile(ids, len(device_ids))
        else:
            rc = lib.axon_start_nrt_profile(None, 0)
        if rc != 0:
            raise RuntimeError(f"axon_start_nrt_profile rc={rc}")
        try:
            yield
        finally:
            n = lib.axon_stop_nrt_profile(str(output_dir).encode())
            print(f"ntff profile: {n} file(s) written to {output_dir}")

    mod = types.ModuleType("antenv.axon_hooks")
    mod.get_axon_ntff_profile_hook = lambda: _hook  # type: ignore[attr-defined]
    mod.set_axon_ntff_profile_hook = lambda h: None  # type: ignore[attr-defined]
    sys.modules["antenv.axon_hooks"] = mod


def run(trace=False, **inputs):
    nc = _build()
    maps = _prep_maps(**inputs)
    if trace:
        _install_ntff_hook()
    res = run_bass_kernel_spmd(nc, maps, list(range(N_CORES)), trace=trace)
    return _gather(res.results), res


def kernel(**inputs) -> np.ndarray:
    out, _ = run(trace=False, **inputs)
    return out
